# revision 15
# baseline (speedup 1.0000x reference)
"""Differential attention kernel for 8 Trainium2 NeuronCores.

Reference computation (per batch b, output head h, with score heads 2h, 2h+1):
    S_i = q[b,2h+i] @ k[b,2h+i].T * (1/8), causal-masked, softmax -> P_i
    y[b,h] = RMSNorm(P_1 @ v - lambda_h * P_2 @ v) * (1 - lambda_init)

Sharding: the 64 (b, h) head-pairs are split 8 per core (data + head parallel).
Lambda params / rms weight are replicated (lambda reduced host-side to the
per-head scalar the reference computes).

v2 design notes (per head-pair, T=1024, d=64, vd=128, 128-row tiles):
  - scores TRANSPOSED: S^T[s, q] = k~.T @ q~ (d-major operands from host);
    the two score heads pack into the top/bottom 64-row halves of the PE
    array and run concurrently.
  - P~ tiles live in ONE causal-packed SBUF tile per pair [128, 2, 4608]
    (s-tile j occupies cols OFF[j]..OFF[j]+1024-128j). exp runs on fixed
    512-score-col chunks (9 per pair) so every ACT call is 1024 elements -
    the ~293ns-per-call ACT overhead amortizes.
  - optional bit-trick exp on DVE for some chunks: fp16(x) bits =
    round(x*SCALING*1024/ln2 + 15316) computed as one tensor_scalar into an
    int16 view of the P~ tile (Schraudolph; ~±3% worst-case, spent only on
    a minority of chunks to stay inside the error budget).
  - causal diagonal masked post-exp by affine_select (gpsimd) or a const
    0/1-triangle multiply (DVE), both heads in one op.
  - PV accumulates three q-tiles per PSUM bank ([128, 3, 129], Y1/Y2 in
    separate banks; col 128 = softmax denominator via the ones-column of V)
    so the whole epilogue batches: one reciprocal + one STT for
    r = -lam*s1/s2 per group, one fused STT combine z = Y1 + r*Y2 per
    q-tile (fp16 out), one fp16 square-accumulate STT per q-tile.
  - RMSNorm is scale-invariant, so z is normalized directly; rsqrt via
    exp(-0.5*ln(x)) keeps ACT on one table set.
  - v is uploaded pre-tiled [128, 8, 128] fp16 so its DMA is contiguous
    (the v1 strided layout ran the SWDGE queues at ~6 GB/s).
"""

import contextlib
import ctypes
import math
import sys
import types
from contextlib import ExitStack

if "/opt/trn_rl_repo" not in sys.path:
    sys.path.insert(0, "/opt/trn_rl_repo")

import numpy as np


# ---------------------------------------------------------------------------
# antenv.axon_hooks shim: the agent image's antenv lacks axon_hooks, which
# concourse.bass_utils hard-imports when trace=True under axon. Recreate the
# module and register the same ctypes NTFF hook trn_boot would have.
def _install_axon_ntff_shim():
    if "antenv.axon_hooks" in sys.modules:
        return
    mod = types.ModuleType("antenv.axon_hooks")
    mod._hook = None
    mod.set_axon_ntff_profile_hook = lambda h: setattr(mod, "_hook", h)
    mod.get_axon_ntff_profile_hook = lambda: mod._hook
    sys.modules["antenv.axon_hooks"] = mod
    try:
        import antenv

        antenv.axon_hooks = mod
    except ImportError:
        pass
    try:
        lib = ctypes.CDLL("/opt/axon/libaxon_pjrt.so")
    except OSError:
        return
    if not hasattr(lib, "axon_start_nrt_profile"):
        return
    lib.axon_start_nrt_profile.argtypes = [
        ctypes.POINTER(ctypes.c_int64),
        ctypes.c_size_t,
    ]
    lib.axon_start_nrt_profile.restype = ctypes.c_int64
    lib.axon_stop_nrt_profile.argtypes = [ctypes.c_char_p]
    lib.axon_stop_nrt_profile.restype = ctypes.c_int64

    @contextlib.contextmanager
    def _hook(output_dir, device_ids):
        import jax

        jax.devices()
        if device_ids:
            ids = (ctypes.c_int64 * len(device_ids))(*device_ids)
            rc = lib.axon_start_nrt_profile(ids, len(device_ids))
        else:
            rc = lib.axon_start_nrt_profile(None, 0)
        if rc != 0:
            raise RuntimeError(f"axon_start_nrt_profile rc={rc}")
        try:
            yield
        finally:
            n = lib.axon_stop_nrt_profile(str(output_dir).encode())
            if n < 0:
                raise RuntimeError(f"axon_stop_nrt_profile rc={n}")

    mod.set_axon_ntff_profile_hook(_hook)


_install_axon_ntff_shim()

import concourse.bass as bass  # noqa: E402
import concourse.mybir as mybir  # noqa: E402
import concourse.tile as tile  # noqa: E402
from concourse import bacc, bass_utils  # noqa: E402
from concourse.alu_op_type import AluOpType  # noqa: E402

# Problem constants (hardcoded per the harness contract).
N_HEADS = 16
D_HEAD = 64
DEPTH = 12
LAMBDA_INIT = 0.8 - 0.6 * math.exp(-0.3 * DEPTH)
SCALING = 1.0 / math.sqrt(D_HEAD)
RMS_EPS = 1e-6
B, T = 4, 1024
CFAC = 1.0 - LAMBDA_INIT

N_CORES = 8
PAIRS = (B * N_HEADS) // N_CORES  # head-pairs per core = 8
BLK = 128
NJ = T // BLK  # 8 s/q tiles

# --- causal-packed score layout -------------------------------------------
W_J = [T - BLK * j for j in range(NJ)]  # width of s-tile j (q >= 128j)
OFF_J = [sum(W_J[:j]) for j in range(NJ)]  # col offset in the packed tile
TOT = sum(W_J)  # 4608
CHUNK = 512
NCH = TOT // CHUNK  # 9

# chunk k -> list of (j, c0_within_j, width, dst_off_within_chunk)
SEGS = []
for k in range(NCH):
    lo, hi = CHUNK * k, CHUNK * (k + 1)
    segs = []
    for j in range(NJ):
        a, b_ = max(lo, OFF_J[j]), min(hi, OFF_J[j] + W_J[j])
        if a < b_:
            segs.append((j, a - OFF_J[j], b_ - a, a - lo))
    SEGS.append(segs)
# chunk that completes s-tile j's diagonal block (cols OFF_J[j]..+128)
DIAG_CHUNK = [(OFF_J[j] + BLK - 1) // CHUNK for j in range(NJ)]

# PV q-tile groups: (start, end) q-tiles; each group = one PSUM bank per head
GROUPS = [(0, 3), (3, 6), (6, 8)]
# group g may start once chunks 0..NEED_CHUNK[g] have been exp'd
NEED_CHUNK = []
for (a, b_) in GROUPS:
    need = 0
    for i in range(a, b_):
        for j in range(i + 1):
            c0 = OFF_J[j] + BLK * (i - j)
            need = max(need, (c0 + BLK - 1) // CHUNK)
    NEED_CHUNK.append(need)

# step schedule: chunks 0..8 with PV groups interleaved as soon as ready
STEPS = []
_g = 0
for k in range(NCH):
    STEPS.append(("c", k))
    while _g < len(GROUPS) and NEED_CHUNK[_g] == k:
        STEPS.append(("g", _g))
        _g += 1
while _g < len(GROUPS):
    STEPS.append(("g", _g))
    _g += 1
NSTEPS = len(STEPS)  # 12

# --- engine assignment knobs ----------------------------------------------
# chunks whose exp runs as the DVE bit-trick instead of ACT (per pair-parity)
DVE_EXP_CHUNKS = frozenset()  # e.g. {1, 5}
# causal-diag mask engine: "gpsimd", "dve", or "split" (by pair parity)
MASK_ENGINE = "gpsimd"
DUAL_PSUM_STT = False  # illegal on HW: an op may read only ONE PSUM input
LAG = 3  # lane B stagger, in steps

A_TRICK = SCALING * 1024.0 / math.log(2.0)
B_TRICK = 15316.0


def _kernel_body(tc, qk_ap, v_ap, lamn_ap, wv_ap, out_ap):
    nc = tc.nc
    f32 = mybir.dt.float32
    f16 = mybir.dt.float16
    i16 = mybir.dt.int16
    Exp = mybir.ActivationFunctionType.Exp
    Ln = mybir.ActivationFunctionType.Ln

    with ExitStack() as ctx:
        const = ctx.enter_context(tc.tile_pool(name="const", bufs=1))
        qkp = ctx.enter_context(tc.tile_pool(name="qkp", bufs=6))
        vp = ctx.enter_context(tc.tile_pool(name="vp", bufs=4))
        ptp = ctx.enter_context(tc.tile_pool(name="ptp", bufs=3))
        scp = ctx.enter_context(tc.tile_pool(name="scp", bufs=2, space="PSUM"))
        yp = ctx.enter_context(tc.tile_pool(name="yp", bufs=2, space="PSUM"))
        zsp = ctx.enter_context(tc.tile_pool(name="zsp", bufs=PAIRS))
        z2p = ctx.enter_context(tc.tile_pool(name="z2p", bufs=2))
        smp = ctx.enter_context(tc.tile_pool(name="smp", bufs=4))
        stp = ctx.enter_context(tc.tile_pool(name="stp", bufs=2))
        tmpp = ctx.enter_context(tc.tile_pool(name="tmpp", bufs=2))
        outp = ctx.enter_context(tc.tile_pool(name="outp", bufs=4))



        # -lambda per pair, broadcast across partitions.
        lamn_sb = const.tile([BLK, PAIRS], f32)
        nc.gpsimd.dma_start(out=lamn_sb, in_=lamn_ap.partition_broadcast(BLK))
        wv_sb = None
        if wv_ap is not None:
            wv_sb = const.tile([BLK, BLK], f32)
            nc.gpsimd.dma_start(out=wv_sb, in_=wv_ap.partition_broadcast(BLK))

        # 0/1 lower-triangle constant for DVE-side causal masking
        tri_t = None
        if MASK_ENGINE in ("dve", "split"):
            tri_t = const.tile([BLK, BLK], f16)
            nc.gpsimd.memset(tri_t, 1.0)
            nc.gpsimd.affine_select(
                out=tri_t, in_=tri_t, compare_op=AluOpType.is_ge, fill=0.0,
                base=0, pattern=[[1, BLK]], channel_multiplier=-1)

        # All pairs' sum-of-squares stats in one tile so the RMSNorm
        # ln/exp chain runs in (at most) two batches.
        stats_all = const.tile([BLK, PAIRS * NJ], f32)
        rs_all = const.tile([BLK, PAIRS * NJ], f32)
        zs_all = [None] * PAIRS

        def emit_rsqrt(dst, src, eng=None):
            """dst = CFAC * rsqrt(src/128 + eps) on DVE only (no ACT table
            traffic): Quake-style int-domain seed (the >>1 done as a *0.5
            float multiply on the int value - exact enough) + two Newton
            steps; CFAC folded into the last step's constants."""
            eng = eng or nc.vector
            n = dst.shape[1]
            m = stp.tile([BLK, n], f32, tag="m")
            eng.tensor_scalar(
                out=m, in0=src, scalar1=1.0 / BLK, scalar2=RMS_EPS,
                op0=AluOpType.mult, op1=AluOpType.add)
            r0 = stp.tile([BLK, n], f32, tag="r0")
            eng.tensor_scalar(
                out=r0.bitcast(mybir.dt.int32), in0=m.bitcast(mybir.dt.int32),
                scalar1=-0.5, scalar2=1597463007.0,
                op0=AluOpType.mult, op1=AluOpType.add)
            t = stp.tile([BLK, n], f32, tag="t")
            for it in range(2):
                eng.tensor_tensor(out=t, in0=r0, in1=r0, op=AluOpType.mult)
                eng.tensor_tensor(out=t, in0=t, in1=m, op=AluOpType.mult)
                cf = CFAC if it == 1 else 1.0
                eng.tensor_scalar(
                    out=t, in0=t, scalar1=-0.5 * cf, scalar2=1.5 * cf,
                    op0=AluOpType.mult, op1=AluOpType.add)
                eng.tensor_tensor(
                    out=dst if it == 1 else r0, in0=r0, in1=t,
                    op=AluOpType.mult)

        def finalize(p0, p1, a=0, b_=NJ, eng=None):
            """rs = CFAC*rsqrt(mean(z^2)+eps) then o = rs*z for q-tiles
            [a, b_) of pairs [p0, p1); one broadcast-TT per pair."""
            eng = eng or nc.vector
            nw = b_ - a
            if p1 - p0 > 1:
                assert (a, b_) == (0, NJ)
                emit_rsqrt(rs_all[:, NJ * p0:NJ * p1],
                           stats_all[:, NJ * p0:NJ * p1], eng)
            else:
                emit_rsqrt(rs_all[:, NJ * p0 + a:NJ * p0 + b_],
                           stats_all[:, NJ * p0 + a:NJ * p0 + b_], eng)
            for p in range(p0, p1):
                c0, c1 = NJ * p + a, NJ * p + b_
                o_t = outp.tile([BLK, NJ, BLK], f16, tag="o")
                eng.tensor_tensor(
                    out=o_t[:, a:b_, :], in0=zs_all[p][:, a:b_, :],
                    in1=rs_all[:, c0:c1].unsqueeze(2).broadcast_to(
                        [BLK, nw, BLK]),
                    op=AluOpType.mult)
                if wv_sb is not None:
                    nc.vector.tensor_tensor(
                        out=o_t[:, a:b_, :], in0=o_t[:, a:b_, :],
                        in1=wv_sb.unsqueeze(1).broadcast_to([BLK, nw, BLK]),
                        op=AluOpType.mult)
                nc.sync.dma_start(
                    out=out_ap[p].rearrange("(n q) d -> q n d", q=BLK)
                    [:, a:b_, :],
                    in_=o_t[:, a:b_, :])

        class Lane:
            """Per-head-pair emission state."""

            def __init__(self, p):
                self.p = p
                # qq/kk: partitions [64h:64h+64] hold head h's d-major q~/k~.
                # First lane-pair only: split DMAs so chunk 0's matmuls
                # (kk cols 0:128, qq cols 0:512) unblock after 160KB
                # instead of 512KB. Later pairs prefetch whole tiles.
                self.qq_t = qkp.tile([BLK, T], f16, tag="qq")
                self.kk_t = qkp.tile([BLK, T], f16, tag="kk")
                if p < 2:
                    nc.sync.dma_start(out=self.kk_t[:, 0:BLK],
                                      in_=qk_ap[2 * p + 1][:, 0:BLK])
                    nc.sync.dma_start(out=self.qq_t[:, 0:CHUNK],
                                      in_=qk_ap[2 * p][:, 0:CHUNK])
                    nc.sync.dma_start(out=self.kk_t[:, BLK:T],
                                      in_=qk_ap[2 * p + 1][:, BLK:T])
                    nc.sync.dma_start(out=self.qq_t[:, CHUNK:T],
                                      in_=qk_ap[2 * p][:, CHUNK:T])
                else:
                    nc.sync.dma_start(out=self.kk_t, in_=qk_ap[2 * p + 1])
                    nc.sync.dma_start(out=self.qq_t, in_=qk_ap[2 * p])
                self.v_t = None
                self.pt = ptp.tile([BLK, 2, TOT], f16, tag="pt")
                self.zs = zsp.tile([BLK, NJ, BLK], f16, tag="zs")
                zs_all[p] = self.zs

            def step(self, t):
                if t == 1 and self.v_t is None:
                    # deferred so pair 0's qk DMAs own the engines at t=0
                    self.v_t = vp.tile([BLK, NJ, 132], f16, tag="v")
                    nc.gpsimd.dma_start(out=self.v_t[:, :, 0:128],
                                        in_=v_ap[self.p])
                    nc.vector.memset(self.v_t[:, :, 128:129], 1.0)
                kind, arg = STEPS[t]
                if kind == "c":
                    self.emit_chunk(arg)
                else:
                    self.emit_group(arg)

            def emit_chunk(lane, k):
                """QK matmuls + exp (+ diag masks) for packed chunk k."""
                sc = scp.tile([BLK, 2, CHUNK], f32, tag="sc")
                for (j, c0, w, do) in SEGS[k]:
                    for h in range(2):
                        lhsT = lane.kk_t[64 * h:64 * h + 64,
                                         BLK * j:BLK * j + BLK]
                        rhs = lane.qq_t[64 * h:64 * h + 64,
                                        BLK * j + c0:BLK * j + c0 + w]
                        nc.tensor.matmul(
                            sc[:, h, do:do + w], lhsT, rhs,
                            start=True, stop=True,
                            tile_position=(64 * h, 0))
                dst = lane.pt[:, :, CHUNK * k:CHUNK * (k + 1)]
                if k in DVE_EXP_CHUNKS:
                    nc.vector.tensor_scalar(
                        out=dst.bitcast(i16), in0=sc,
                        scalar1=A_TRICK, scalar2=B_TRICK,
                        op0=AluOpType.mult, op1=AluOpType.add)
                else:
                    nc.scalar.activation(out=dst, in_=sc, func=Exp,
                                         scale=SCALING)
                for j in range(NJ):
                    if DIAG_CHUNK[j] != k:
                        continue
                    dg = lane.pt[:, :, OFF_J[j]:OFF_J[j] + BLK]
                    use_dve = MASK_ENGINE == "dve" or (
                        MASK_ENGINE == "split" and lane.p % 2 == 1)
                    if use_dve:
                        nc.vector.tensor_tensor(
                            out=dg, in0=dg,
                            in1=tri_t.unsqueeze(1).broadcast_to([BLK, 2, BLK]),
                            op=AluOpType.mult)
                    else:
                        nc.gpsimd.affine_select(
                            out=dg, in_=dg, compare_op=AluOpType.is_ge,
                            fill=0.0, base=0, pattern=[[0, 2], [1, BLK]],
                            channel_multiplier=-1)

            def emit_group(lane, g):
                """PV accumulation + epilogue for q-tile group g."""
                a, b_ = GROUPS[g]
                ng = b_ - a
                # one PSUM bank per head: [128, 512] viewed as 3 x 129-col
                # q-tile slots (col 128 of each slot = softmax denominator)
                yb1t = yp.tile([BLK, 512], f32, tag="y1")
                yb2t = yp.tile([BLK, 512], f32, tag="y2")
                yb1 = yb1t[:, 0:387].rearrange("p (t c) -> p t c", c=129)
                yb2 = yb2t[:, 0:387].rearrange("p (t c) -> p t c", c=129)
                for i in range(a, b_):
                    for j in range(i + 1):
                        col = OFF_J[j] + BLK * (i - j)
                        for h, yb in ((0, yb1), (1, yb2)):
                            nc.tensor.matmul(
                                yb[:, i - a, :],
                                lane.pt[:, h, col:col + BLK],
                                lane.v_t[:, j, 0:129],
                                start=(i == a and j == 0),
                                stop=(j == i),
                                skip_group_check=True)
                # r = -lam * s1 / s2 per q-tile of the group
                rec = smp.tile([BLK, 3], f32, tag="rec")
                nc.vector.reciprocal(rec[:, 0:ng],
                                     yb2[:, 0:ng, 128:129].squeeze(2))
                r_t = smp.tile([BLK, 3], f32, tag="r")
                nc.vector.scalar_tensor_tensor(
                    out=r_t[:, 0:ng], in0=rec[:, 0:ng],
                    scalar=lamn_sb[:, lane.p:lane.p + 1],
                    in1=yb1[:, 0:ng, 128:129].squeeze(2),
                    op0=AluOpType.mult, op1=AluOpType.mult)
                # z = Y1 + r*Y2 (fp16)
                if DUAL_PSUM_STT:
                    for tq in range(ng):
                        nc.vector.scalar_tensor_tensor(
                            out=lane.zs[:, a + tq, :],
                            in0=yb2[:, tq, 0:128],
                            scalar=r_t[:, tq:tq + 1],
                            in1=yb1[:, tq, 0:128],
                            op0=AluOpType.mult, op1=AluOpType.add)
                else:
                    tmp = tmpp.tile([BLK, 3, BLK], f16, tag="tmp")
                    nc.vector.tensor_tensor(
                        out=tmp[:, 0:ng], in0=yb2[:, 0:ng, 0:128],
                        in1=r_t[:, 0:ng].unsqueeze(2).broadcast_to([BLK, ng, BLK]),
                        op=AluOpType.mult)
                    nc.vector.tensor_tensor(
                        out=lane.zs[:, a:b_, :], in0=tmp[:, 0:ng],
                        in1=yb1[:, 0:ng, 0:128], op=AluOpType.add)
                # stats += sum(z^2) per q-tile
                for tq in range(ng):
                    z2 = z2p.tile([BLK, BLK], f16, tag="z2")
                    nc.vector.scalar_tensor_tensor(
                        out=z2, in0=lane.zs[:, a + tq, :], scalar=1.0,
                        in1=lane.zs[:, a + tq, :],
                        op0=AluOpType.bypass, op1=AluOpType.mult,
                        accum_out=stats_all[:, NJ * lane.p + a + tq:
                                            NJ * lane.p + a + tq + 1])
                # Last two pairs: finalize in two batches (tiles 0-6
                # after group 1, 6-8 after group 2) so the tail is short;
                # the second-to-last pair's chain runs on GPSIMD so both
                # pairs' finalizes overlap instead of serializing on DVE.
                if lane.p >= PAIRS - 2 and g >= 1:
                    te = nc.gpsimd if lane.p == PAIRS - 2 else nc.vector
                    fa, fb = (0, 6) if g == 1 else (6, NJ)
                    finalize(lane.p, lane.p + 1, fa, fb, eng=te)

        assert PAIRS % 2 == 0
        for grp in range(PAIRS // 2):
            laneA = Lane(2 * grp)
            laneB = Lane(2 * grp + 1)
            for t in range(NSTEPS + LAG):
                if t < NSTEPS:
                    laneA.step(t)
                if 0 <= t - LAG < NSTEPS:
                    laneB.step(t - LAG)
                # Finalize earlier pairs while the last group computes.
                if grp == PAIRS // 2 - 1 and t == 3:
                    finalize(0, PAIRS - 2)


def build_program(apply_weight=False, num_devices=N_CORES):
    nc = bacc.Bacc(
        "TRN2", target_bir_lowering=False, debug=False,
        num_devices=num_devices
    )
    f16 = mybir.dt.float16
    qk_d = nc.dram_tensor("qk", [2 * PAIRS, BLK, T], f16, kind="ExternalInput")
    v_d = nc.dram_tensor("v", [PAIRS, BLK, NJ, BLK], f16, kind="ExternalInput")
    lamn_d = nc.dram_tensor("lamn", [PAIRS], mybir.dt.float32,
                            kind="ExternalInput")
    wv_d = None
    if apply_weight:
        wv_d = nc.dram_tensor("wv", [BLK], mybir.dt.float32,
                              kind="ExternalInput")
    out_d = nc.dram_tensor("out", [PAIRS, T, BLK], f16, kind="ExternalOutput")
    with tile.TileContext(nc) as tc:
        _kernel_body(
            tc,
            qk_d.ap(),
            v_d.ap(),
            lamn_d.ap(),
            wv_d.ap() if wv_d is not None else None,
            out_d.ap(),
        )
    nc.compile()
    return nc


def make_in_maps(q, k, v, lambda_q1, lambda_k1, lambda_q2, lambda_k2,
                 rms_weight):
    """Host-side shard + layout prep. Returns (in_maps, apply_weight)."""
    q = np.ascontiguousarray(
        np.asarray(q, np.float32).transpose(0, 1, 3, 2)).astype(np.float16)
    k = np.ascontiguousarray(
        np.asarray(k, np.float32).transpose(0, 1, 3, 2)).astype(np.float16)
    v = np.asarray(v, np.float32)
    lq1 = np.asarray(lambda_q1, np.float64)
    lk1 = np.asarray(lambda_k1, np.float64)
    lq2 = np.asarray(lambda_q2, np.float64)
    lk2 = np.asarray(lambda_k2, np.float64)
    lam1 = np.exp(np.sum(lq1 * lk1, axis=-1))
    lam2 = np.exp(np.sum(lq2 * lk2, axis=-1))
    lam = (lam1 - lam2 + LAMBDA_INIT).astype(np.float32)  # [N_HEADS]
    w = np.asarray(rms_weight, np.float32)
    apply_weight = not np.all(w == 1.0)

    # v pre-tiled to [128 s-part, NJ, 128] fp16 so the device DMA is
    # contiguous: v_c[p, s, n, d] = v[b, h, 128n + s, d]
    in_maps = []
    for c in range(N_CORES):
        qk_c = np.empty((2 * PAIRS, BLK, T), np.float16)
        v_c = np.empty((PAIRS, BLK, NJ, BLK), np.float16)
        lamn_c = np.empty((PAIRS,), np.float32)
        for p in range(PAIRS):
            g = c * PAIRS + p
            b, h = divmod(g, N_HEADS)
            qk_c[2 * p, 0:64] = q[b, 2 * h]
            qk_c[2 * p, 64:128] = q[b, 2 * h + 1]
            qk_c[2 * p + 1, 0:64] = k[b, 2 * h]
            qk_c[2 * p + 1, 64:128] = k[b, 2 * h + 1]
            v_c[p] = v[b, h].reshape(NJ, BLK, BLK).transpose(1, 0, 2)
            lamn_c[p] = -lam[h]
        m = {"qk": qk_c, "v": v_c, "lamn": lamn_c}
        if apply_weight:
            m["wv"] = w
        in_maps.append(m)
    return in_maps, apply_weight


def kernel(q, k, v, mask, lambda_q1, lambda_k1, lambda_q2, lambda_k2,
           rms_weight, flash_attn=0, _trace=False, _nc_cache={}):
    in_maps, apply_weight = make_in_maps(
        q, k, v, lambda_q1, lambda_k1, lambda_q2, lambda_k2, rms_weight
    )
    key = apply_weight
    if key not in _nc_cache:
        _nc_cache[key] = build_program(apply_weight=apply_weight)
    nc = _nc_cache[key]
    res = bass_utils.run_bass_kernel_spmd(
        nc, in_maps, core_ids=list(range(N_CORES)), trace=_trace
    )
    out = np.empty((B, N_HEADS, T, 2 * D_HEAD), np.float32)
    for c in range(N_CORES):
        oc = res.results[c]["out"].astype(np.float32)
        for p in range(PAIRS):
            g = c * PAIRS + p
            b, h = divmod(g, N_HEADS)
            out[b, h] = oc[p]
    if _trace:
        kernel._last_exec_time_ns = res.exec_time_ns
        kernel._last_results = res
    return out


# revision 16
# speedup vs baseline: 1.2397x; 1.2397x over previous
"""Differential attention kernel for 8 Trainium2 NeuronCores.

Reference computation (per batch b, output head h, with score heads 2h, 2h+1):
    S_i = q[b,2h+i] @ k[b,2h+i].T * (1/8), causal-masked, softmax -> P_i
    y[b,h] = RMSNorm(P_1 @ v - lambda_h * P_2 @ v) * (1 - lambda_init)

Sharding: the 64 (b, h) head-pairs are split 8 per core (data + head parallel).
Lambda params / rms weight are replicated (lambda reduced host-side to the
per-head scalar the reference computes).

v2 design notes (per head-pair, T=1024, d=64, vd=128, 128-row tiles):
  - scores TRANSPOSED: S^T[s, q] = k~.T @ q~ (d-major operands from host);
    the two score heads pack into the top/bottom 64-row halves of the PE
    array and run concurrently.
  - P~ tiles live in ONE causal-packed SBUF tile per pair [128, 2, 4608]
    (s-tile j occupies cols OFF[j]..OFF[j]+1024-128j). exp runs on fixed
    512-score-col chunks (9 per pair) so every ACT call is 1024 elements -
    the ~293ns-per-call ACT overhead amortizes.
  - optional bit-trick exp on DVE for some chunks: fp16(x) bits =
    round(x*SCALING*1024/ln2 + 15316) computed as one tensor_scalar into an
    int16 view of the P~ tile (Schraudolph; ~±3% worst-case, spent only on
    a minority of chunks to stay inside the error budget).
  - causal diagonal masked post-exp by affine_select (gpsimd) or a const
    0/1-triangle multiply (DVE), both heads in one op.
  - PV accumulates three q-tiles per PSUM bank ([128, 3, 129], Y1/Y2 in
    separate banks; col 128 = softmax denominator via the ones-column of V)
    so the whole epilogue batches: one reciprocal + one STT for
    r = -lam*s1/s2 per group, one fused STT combine z = Y1 + r*Y2 per
    q-tile (fp16 out), one fp16 square-accumulate STT per q-tile.
  - RMSNorm is scale-invariant, so z is normalized directly; rsqrt via
    exp(-0.5*ln(x)) keeps ACT on one table set.
  - v is uploaded pre-tiled [128, 8, 128] fp16 so its DMA is contiguous
    (the v1 strided layout ran the SWDGE queues at ~6 GB/s).
"""

import contextlib
import ctypes
import math
import sys
import types
from contextlib import ExitStack

if "/opt/trn_rl_repo" not in sys.path:
    sys.path.insert(0, "/opt/trn_rl_repo")

import numpy as np


# ---------------------------------------------------------------------------
# antenv.axon_hooks shim: the agent image's antenv lacks axon_hooks, which
# concourse.bass_utils hard-imports when trace=True under axon. Recreate the
# module and register the same ctypes NTFF hook trn_boot would have.
def _install_axon_ntff_shim():
    if "antenv.axon_hooks" in sys.modules:
        return
    mod = types.ModuleType("antenv.axon_hooks")
    mod._hook = None
    mod.set_axon_ntff_profile_hook = lambda h: setattr(mod, "_hook", h)
    mod.get_axon_ntff_profile_hook = lambda: mod._hook
    sys.modules["antenv.axon_hooks"] = mod
    try:
        import antenv

        antenv.axon_hooks = mod
    except ImportError:
        pass
    try:
        lib = ctypes.CDLL("/opt/axon/libaxon_pjrt.so")
    except OSError:
        return
    if not hasattr(lib, "axon_start_nrt_profile"):
        return
    lib.axon_start_nrt_profile.argtypes = [
        ctypes.POINTER(ctypes.c_int64),
        ctypes.c_size_t,
    ]
    lib.axon_start_nrt_profile.restype = ctypes.c_int64
    lib.axon_stop_nrt_profile.argtypes = [ctypes.c_char_p]
    lib.axon_stop_nrt_profile.restype = ctypes.c_int64

    @contextlib.contextmanager
    def _hook(output_dir, device_ids):
        import jax

        jax.devices()
        if device_ids:
            ids = (ctypes.c_int64 * len(device_ids))(*device_ids)
            rc = lib.axon_start_nrt_profile(ids, len(device_ids))
        else:
            rc = lib.axon_start_nrt_profile(None, 0)
        if rc != 0:
            raise RuntimeError(f"axon_start_nrt_profile rc={rc}")
        try:
            yield
        finally:
            n = lib.axon_stop_nrt_profile(str(output_dir).encode())
            if n < 0:
                raise RuntimeError(f"axon_stop_nrt_profile rc={n}")

    mod.set_axon_ntff_profile_hook(_hook)


_install_axon_ntff_shim()

import concourse.bass as bass  # noqa: E402
import concourse.mybir as mybir  # noqa: E402
import concourse.tile as tile  # noqa: E402
from concourse import bacc, bass_utils  # noqa: E402
from concourse.alu_op_type import AluOpType  # noqa: E402

# Problem constants (hardcoded per the harness contract).
N_HEADS = 16
D_HEAD = 64
DEPTH = 12
LAMBDA_INIT = 0.8 - 0.6 * math.exp(-0.3 * DEPTH)
SCALING = 1.0 / math.sqrt(D_HEAD)
RMS_EPS = 1e-6
B, T = 4, 1024
CFAC = 1.0 - LAMBDA_INIT

N_CORES = 8
PAIRS = (B * N_HEADS) // N_CORES  # head-pairs per core = 8
BLK = 128
NJ = T // BLK  # 8 s/q tiles

# --- causal-packed score layout -------------------------------------------
W_J = [T - BLK * j for j in range(NJ)]  # width of s-tile j (q >= 128j)
OFF_J = [sum(W_J[:j]) for j in range(NJ)]  # col offset in the packed tile
TOT = sum(W_J)  # 4608
CHUNK = 512
NCH = TOT // CHUNK  # 9

# chunk k -> list of (j, c0_within_j, width, dst_off_within_chunk)
SEGS = []
for k in range(NCH):
    lo, hi = CHUNK * k, CHUNK * (k + 1)
    segs = []
    for j in range(NJ):
        a, b_ = max(lo, OFF_J[j]), min(hi, OFF_J[j] + W_J[j])
        if a < b_:
            segs.append((j, a - OFF_J[j], b_ - a, a - lo))
    SEGS.append(segs)
# chunk that completes s-tile j's diagonal block (cols OFF_J[j]..+128)
DIAG_CHUNK = [(OFF_J[j] + BLK - 1) // CHUNK for j in range(NJ)]

# PV q-tile groups: (start, end) q-tiles; each group = one PSUM bank per head
GROUPS = [(0, 3), (3, 6), (6, 8)]
# group g may start once chunks 0..NEED_CHUNK[g] have been exp'd
NEED_CHUNK = []
for (a, b_) in GROUPS:
    need = 0
    for i in range(a, b_):
        for j in range(i + 1):
            c0 = OFF_J[j] + BLK * (i - j)
            need = max(need, (c0 + BLK - 1) // CHUNK)
    NEED_CHUNK.append(need)

# step schedule: chunks 0..8 with PV groups interleaved as soon as ready
STEPS = []
_g = 0
for k in range(NCH):
    STEPS.append(("c", k))
    while _g < len(GROUPS) and NEED_CHUNK[_g] == k:
        STEPS.append(("g", _g))
        _g += 1
while _g < len(GROUPS):
    STEPS.append(("g", _g))
    _g += 1
NSTEPS = len(STEPS)  # 12

# --- engine assignment knobs ----------------------------------------------
# chunks whose exp runs as the DVE bit-trick instead of ACT (per pair-parity)
DVE_EXP_CHUNKS = frozenset()  # e.g. {1, 5}
# causal-diag mask engine: "gpsimd", "dve", or "split" (by pair parity)
MASK_ENGINE = "gpsimd"
DUAL_PSUM_STT = False  # illegal on HW: an op may read only ONE PSUM input
LAG = 3  # lane B stagger, in steps

A_TRICK = SCALING * 1024.0 / math.log(2.0)
B_TRICK = 15316.0


def _kernel_body(tc, qk_ap, v_ap, lamn_ap, wv_ap, out_ap):
    nc = tc.nc
    f32 = mybir.dt.float32
    f16 = mybir.dt.float16
    i16 = mybir.dt.int16
    Exp = mybir.ActivationFunctionType.Exp
    Ln = mybir.ActivationFunctionType.Ln

    with ExitStack() as ctx:
        const = ctx.enter_context(tc.tile_pool(name="const", bufs=1))
        qkp = ctx.enter_context(tc.tile_pool(name="qkp", bufs=6))
        vp = ctx.enter_context(tc.tile_pool(name="vp", bufs=4))
        ptp = ctx.enter_context(tc.tile_pool(name="ptp", bufs=3))
        scp = ctx.enter_context(tc.tile_pool(name="scp", bufs=2, space="PSUM"))
        yp = ctx.enter_context(tc.tile_pool(name="yp", bufs=2, space="PSUM"))
        zsp = ctx.enter_context(tc.tile_pool(name="zsp", bufs=PAIRS))
        z2p = ctx.enter_context(tc.tile_pool(name="z2p", bufs=2))
        smp = ctx.enter_context(tc.tile_pool(name="smp", bufs=4))
        stp = ctx.enter_context(tc.tile_pool(name="stp", bufs=2))
        tmpp = ctx.enter_context(tc.tile_pool(name="tmpp", bufs=2))
        outp = ctx.enter_context(tc.tile_pool(name="outp", bufs=4))



        # -lambda per pair, broadcast across partitions.
        lamn_sb = const.tile([BLK, PAIRS], f32)
        nc.gpsimd.dma_start(out=lamn_sb, in_=lamn_ap.partition_broadcast(BLK))
        wv_sb = None
        if wv_ap is not None:
            wv_sb = const.tile([BLK, BLK], f32)
            nc.gpsimd.dma_start(out=wv_sb, in_=wv_ap.partition_broadcast(BLK))

        # 0/1 lower-triangle constant for DVE-side causal masking
        tri_t = None
        if MASK_ENGINE in ("dve", "split"):
            tri_t = const.tile([BLK, BLK], f16)
            nc.gpsimd.memset(tri_t, 1.0)
            nc.gpsimd.affine_select(
                out=tri_t, in_=tri_t, compare_op=AluOpType.is_ge, fill=0.0,
                base=0, pattern=[[1, BLK]], channel_multiplier=-1)

        # All pairs' sum-of-squares stats in one tile so the RMSNorm
        # ln/exp chain runs in (at most) two batches.
        stats_all = const.tile([BLK, PAIRS * NJ], f32)
        rs_all = const.tile([BLK, PAIRS * NJ], f32)
        zs_all = [None] * PAIRS

        def emit_rsqrt(dst, src, eng=None):
            """dst = CFAC * rsqrt(src/128 + eps) on DVE only (no ACT table
            traffic): Quake-style int-domain seed (the >>1 done as a *0.5
            float multiply on the int value - exact enough) + two Newton
            steps; CFAC folded into the last step's constants."""
            eng = eng or nc.vector
            n = dst.shape[1]
            m = stp.tile([BLK, n], f32, tag="m")
            eng.tensor_scalar(
                out=m, in0=src, scalar1=1.0 / BLK, scalar2=RMS_EPS,
                op0=AluOpType.mult, op1=AluOpType.add)
            r0 = stp.tile([BLK, n], f32, tag="r0")
            eng.tensor_scalar(
                out=r0.bitcast(mybir.dt.int32), in0=m.bitcast(mybir.dt.int32),
                scalar1=-0.5, scalar2=1597463007.0,
                op0=AluOpType.mult, op1=AluOpType.add)
            t = stp.tile([BLK, n], f32, tag="t")
            for it in range(2):
                eng.tensor_tensor(out=t, in0=r0, in1=r0, op=AluOpType.mult)
                eng.tensor_tensor(out=t, in0=t, in1=m, op=AluOpType.mult)
                cf = CFAC if it == 1 else 1.0
                eng.tensor_scalar(
                    out=t, in0=t, scalar1=-0.5 * cf, scalar2=1.5 * cf,
                    op0=AluOpType.mult, op1=AluOpType.add)
                eng.tensor_tensor(
                    out=dst if it == 1 else r0, in0=r0, in1=t,
                    op=AluOpType.mult)

        def finalize(p0, p1, a=0, b_=NJ, eng=None):
            """rs = CFAC*rsqrt(mean(z^2)+eps) then o = rs*z for q-tiles
            [a, b_) of pairs [p0, p1); one broadcast-TT per pair."""
            eng = eng or nc.vector
            nw = b_ - a
            if p1 - p0 > 1:
                assert (a, b_) == (0, NJ)
                emit_rsqrt(rs_all[:, NJ * p0:NJ * p1],
                           stats_all[:, NJ * p0:NJ * p1], eng)
            else:
                emit_rsqrt(rs_all[:, NJ * p0 + a:NJ * p0 + b_],
                           stats_all[:, NJ * p0 + a:NJ * p0 + b_], eng)
            for p in range(p0, p1):
                c0, c1 = NJ * p + a, NJ * p + b_
                o_t = outp.tile([BLK, NJ, BLK], f16, tag="o")
                eng.tensor_tensor(
                    out=o_t[:, a:b_, :], in0=zs_all[p][:, a:b_, :],
                    in1=rs_all[:, c0:c1].unsqueeze(2).broadcast_to(
                        [BLK, nw, BLK]),
                    op=AluOpType.mult)
                if wv_sb is not None:
                    nc.vector.tensor_tensor(
                        out=o_t[:, a:b_, :], in0=o_t[:, a:b_, :],
                        in1=wv_sb.unsqueeze(1).broadcast_to([BLK, nw, BLK]),
                        op=AluOpType.mult)
                nc.sync.dma_start(
                    out=out_ap[p].rearrange("(n q) d -> q n d", q=BLK)
                    [:, a:b_, :],
                    in_=o_t[:, a:b_, :])

        class Lane:
            """Per-head-pair emission state."""

            def __init__(self, p):
                self.p = p
                # qq/kk: partitions [64h:64h+64] hold head h's d-major q~/k~.
                # First lane-pair only: split DMAs so chunk 0's matmuls
                # (kk cols 0:128, qq cols 0:512) unblock after 160KB
                # instead of 512KB. Later pairs prefetch whole tiles.
                self.qq_t = qkp.tile([BLK, T], f16, tag="qq")
                self.kk_t = qkp.tile([BLK, T], f16, tag="kk")
                if p < 2:
                    nc.sync.dma_start(out=self.kk_t[:, 0:BLK],
                                      in_=qk_ap[2 * p + 1][:, 0:BLK])
                    nc.sync.dma_start(out=self.qq_t[:, 0:CHUNK],
                                      in_=qk_ap[2 * p][:, 0:CHUNK])
                    nc.sync.dma_start(out=self.kk_t[:, BLK:T],
                                      in_=qk_ap[2 * p + 1][:, BLK:T])
                    nc.sync.dma_start(out=self.qq_t[:, CHUNK:T],
                                      in_=qk_ap[2 * p][:, CHUNK:T])
                else:
                    nc.sync.dma_start(out=self.kk_t, in_=qk_ap[2 * p + 1])
                    nc.sync.dma_start(out=self.qq_t, in_=qk_ap[2 * p])
                self.v_t = None
                self.pt = ptp.tile([BLK, 2, TOT], f16, tag="pt")
                self.zs = zsp.tile([BLK, NJ, BLK], f16, tag="zs")
                zs_all[p] = self.zs

            def step(self, t):
                if t == 1 and self.v_t is None:
                    # deferred so pair 0's qk DMAs own the engines at t=0
                    self.v_t = vp.tile([BLK, NJ, 132], f16, tag="v")
                    nc.gpsimd.dma_start(out=self.v_t[:, :, 0:128],
                                        in_=v_ap[self.p])
                    nc.vector.memset(self.v_t[:, :, 128:129], 1.0)
                kind, arg = STEPS[t]
                if kind == "c":
                    self.emit_chunk(arg)
                else:
                    self.emit_group(arg)

            def emit_chunk(lane, k):
                """QK matmuls + exp (+ diag masks) for packed chunk k."""
                sc = scp.tile([BLK, 2, CHUNK], f32, tag="sc")
                for (j, c0, w, do) in SEGS[k]:
                    for h in range(2):
                        lhsT = lane.kk_t[64 * h:64 * h + 64,
                                         BLK * j:BLK * j + BLK]
                        rhs = lane.qq_t[64 * h:64 * h + 64,
                                        BLK * j + c0:BLK * j + c0 + w]
                        nc.tensor.matmul(
                            sc[:, h, do:do + w], lhsT, rhs,
                            start=True, stop=True,
                            tile_position=(64 * h, 0))
                dst = lane.pt[:, :, CHUNK * k:CHUNK * (k + 1)]
                if k in DVE_EXP_CHUNKS:
                    nc.vector.tensor_scalar(
                        out=dst.bitcast(i16), in0=sc,
                        scalar1=A_TRICK, scalar2=B_TRICK,
                        op0=AluOpType.mult, op1=AluOpType.add)
                else:
                    nc.scalar.activation(out=dst, in_=sc, func=Exp,
                                         scale=SCALING)
                for j in range(NJ):
                    if DIAG_CHUNK[j] != k:
                        continue
                    dg = lane.pt[:, :, OFF_J[j]:OFF_J[j] + BLK]
                    use_dve = MASK_ENGINE == "dve" or (
                        MASK_ENGINE == "split" and lane.p % 2 == 1)
                    if use_dve:
                        nc.vector.tensor_tensor(
                            out=dg, in0=dg,
                            in1=tri_t.unsqueeze(1).broadcast_to([BLK, 2, BLK]),
                            op=AluOpType.mult)
                    else:
                        nc.gpsimd.affine_select(
                            out=dg, in_=dg, compare_op=AluOpType.is_ge,
                            fill=0.0, base=0, pattern=[[0, 2], [1, BLK]],
                            channel_multiplier=-1)

            def emit_group(lane, g):
                """PV accumulation + epilogue for q-tile group g."""
                a, b_ = GROUPS[g]
                ng = b_ - a
                # one PSUM bank per head: [128, 512] viewed as 3 x 129-col
                # q-tile slots (col 128 of each slot = softmax denominator)
                yb1t = yp.tile([BLK, 512], f32, tag="y1")
                yb2t = yp.tile([BLK, 512], f32, tag="y2")
                yb1 = yb1t[:, 0:387].rearrange("p (t c) -> p t c", c=129)
                yb2 = yb2t[:, 0:387].rearrange("p (t c) -> p t c", c=129)
                for i in range(a, b_):
                    for j in range(i + 1):
                        col = OFF_J[j] + BLK * (i - j)
                        for h, yb in ((0, yb1), (1, yb2)):
                            nc.tensor.matmul(
                                yb[:, i - a, :],
                                lane.pt[:, h, col:col + BLK],
                                lane.v_t[:, j, 0:129],
                                start=(i == a and j == 0),
                                stop=(j == i),
                                skip_group_check=True)
                # r = -lam * s1 / s2 per q-tile of the group
                rec = smp.tile([BLK, 3], f32, tag="rec")
                nc.vector.reciprocal(rec[:, 0:ng],
                                     yb2[:, 0:ng, 128:129].squeeze(2))
                r_t = smp.tile([BLK, 3], f32, tag="r")
                nc.vector.scalar_tensor_tensor(
                    out=r_t[:, 0:ng], in0=rec[:, 0:ng],
                    scalar=lamn_sb[:, lane.p:lane.p + 1],
                    in1=yb1[:, 0:ng, 128:129].squeeze(2),
                    op0=AluOpType.mult, op1=AluOpType.mult)
                # z = Y1 + r*Y2 (fp16)
                if DUAL_PSUM_STT:
                    for tq in range(ng):
                        nc.vector.scalar_tensor_tensor(
                            out=lane.zs[:, a + tq, :],
                            in0=yb2[:, tq, 0:128],
                            scalar=r_t[:, tq:tq + 1],
                            in1=yb1[:, tq, 0:128],
                            op0=AluOpType.mult, op1=AluOpType.add)
                else:
                    tmp = tmpp.tile([BLK, 3, BLK], f16, tag="tmp")
                    nc.vector.tensor_tensor(
                        out=tmp[:, 0:ng], in0=yb2[:, 0:ng, 0:128],
                        in1=r_t[:, 0:ng].unsqueeze(2).broadcast_to([BLK, ng, BLK]),
                        op=AluOpType.mult)
                    nc.vector.tensor_tensor(
                        out=lane.zs[:, a:b_, :], in0=tmp[:, 0:ng],
                        in1=yb1[:, 0:ng, 0:128], op=AluOpType.add)
                # stats += sum(z^2) per q-tile
                for tq in range(ng):
                    z2 = z2p.tile([BLK, BLK], f16, tag="z2")
                    nc.vector.scalar_tensor_tensor(
                        out=z2, in0=lane.zs[:, a + tq, :], scalar=1.0,
                        in1=lane.zs[:, a + tq, :],
                        op0=AluOpType.bypass, op1=AluOpType.mult,
                        accum_out=stats_all[:, NJ * lane.p + a + tq:
                                            NJ * lane.p + a + tq + 1])
                # Last two pairs: finalize in two batches (tiles 0-6
                # after group 1, 6-8 after group 2) so the tail is short;
                # the second-to-last pair's chain runs on GPSIMD so both
                # pairs' finalizes overlap instead of serializing on DVE.
                if lane.p >= PAIRS - 2 and g >= 1:
                    fa, fb = (0, 6) if g == 1 else (6, NJ)
                    finalize(lane.p, lane.p + 1, fa, fb)

        assert PAIRS % 2 == 0
        for grp in range(PAIRS // 2):
            laneA = Lane(2 * grp)
            laneB = Lane(2 * grp + 1)
            for t in range(NSTEPS + LAG):
                if t < NSTEPS:
                    laneA.step(t)
                if 0 <= t - LAG < NSTEPS:
                    laneB.step(t - LAG)
                # Finalize earlier pairs while the last group computes.
                if grp == PAIRS // 2 - 1 and t == 3:
                    finalize(0, PAIRS - 2)


def build_program(apply_weight=False, num_devices=N_CORES):
    nc = bacc.Bacc(
        "TRN2", target_bir_lowering=False, debug=False,
        num_devices=num_devices
    )
    f16 = mybir.dt.float16
    qk_d = nc.dram_tensor("qk", [2 * PAIRS, BLK, T], f16, kind="ExternalInput")
    v_d = nc.dram_tensor("v", [PAIRS, BLK, NJ, BLK], f16, kind="ExternalInput")
    lamn_d = nc.dram_tensor("lamn", [PAIRS], mybir.dt.float32,
                            kind="ExternalInput")
    wv_d = None
    if apply_weight:
        wv_d = nc.dram_tensor("wv", [BLK], mybir.dt.float32,
                              kind="ExternalInput")
    out_d = nc.dram_tensor("out", [PAIRS, T, BLK], f16, kind="ExternalOutput")
    with tile.TileContext(nc) as tc:
        _kernel_body(
            tc,
            qk_d.ap(),
            v_d.ap(),
            lamn_d.ap(),
            wv_d.ap() if wv_d is not None else None,
            out_d.ap(),
        )
    nc.compile()
    return nc


def make_in_maps(q, k, v, lambda_q1, lambda_k1, lambda_q2, lambda_k2,
                 rms_weight):
    """Host-side shard + layout prep. Returns (in_maps, apply_weight)."""
    q = np.ascontiguousarray(
        np.asarray(q, np.float32).transpose(0, 1, 3, 2)).astype(np.float16)
    k = np.ascontiguousarray(
        np.asarray(k, np.float32).transpose(0, 1, 3, 2)).astype(np.float16)
    v = np.asarray(v, np.float32)
    lq1 = np.asarray(lambda_q1, np.float64)
    lk1 = np.asarray(lambda_k1, np.float64)
    lq2 = np.asarray(lambda_q2, np.float64)
    lk2 = np.asarray(lambda_k2, np.float64)
    lam1 = np.exp(np.sum(lq1 * lk1, axis=-1))
    lam2 = np.exp(np.sum(lq2 * lk2, axis=-1))
    lam = (lam1 - lam2 + LAMBDA_INIT).astype(np.float32)  # [N_HEADS]
    w = np.asarray(rms_weight, np.float32)
    apply_weight = not np.all(w == 1.0)

    # v pre-tiled to [128 s-part, NJ, 128] fp16 so the device DMA is
    # contiguous: v_c[p, s, n, d] = v[b, h, 128n + s, d]
    in_maps = []
    for c in range(N_CORES):
        qk_c = np.empty((2 * PAIRS, BLK, T), np.float16)
        v_c = np.empty((PAIRS, BLK, NJ, BLK), np.float16)
        lamn_c = np.empty((PAIRS,), np.float32)
        for p in range(PAIRS):
            g = c * PAIRS + p
            b, h = divmod(g, N_HEADS)
            qk_c[2 * p, 0:64] = q[b, 2 * h]
            qk_c[2 * p, 64:128] = q[b, 2 * h + 1]
            qk_c[2 * p + 1, 0:64] = k[b, 2 * h]
            qk_c[2 * p + 1, 64:128] = k[b, 2 * h + 1]
            v_c[p] = v[b, h].reshape(NJ, BLK, BLK).transpose(1, 0, 2)
            lamn_c[p] = -lam[h]
        m = {"qk": qk_c, "v": v_c, "lamn": lamn_c}
        if apply_weight:
            m["wv"] = w
        in_maps.append(m)
    return in_maps, apply_weight


def kernel(q, k, v, mask, lambda_q1, lambda_k1, lambda_q2, lambda_k2,
           rms_weight, flash_attn=0, _trace=False, _nc_cache={}):
    in_maps, apply_weight = make_in_maps(
        q, k, v, lambda_q1, lambda_k1, lambda_q2, lambda_k2, rms_weight
    )
    key = apply_weight
    if key not in _nc_cache:
        _nc_cache[key] = build_program(apply_weight=apply_weight)
    nc = _nc_cache[key]
    res = bass_utils.run_bass_kernel_spmd(
        nc, in_maps, core_ids=list(range(N_CORES)), trace=_trace
    )
    out = np.empty((B, N_HEADS, T, 2 * D_HEAD), np.float32)
    for c in range(N_CORES):
        oc = res.results[c]["out"].astype(np.float32)
        for p in range(PAIRS):
            g = c * PAIRS + p
            b, h = divmod(g, N_HEADS)
            out[b, h] = oc[p]
    if _trace:
        kernel._last_exec_time_ns = res.exec_time_ns
        kernel._last_results = res
    return out
